# revision 3
# baseline (speedup 1.0000x reference)
"""2-layer GCN encoder on 8 Trainium2 NeuronCores (Bass/Tile).

Sharding: dst-nodes partitioned across 8 cores (12,500 each, padded to
12,544 = 98*128). Weights replicated. Per layer:
  1. transform: g = dinv * (x @ W) per 128-node tile on the TensorEngine
  2. AllGather bf16 g shards -> full table in DRAM (halo exchange)
  3. dma_gather of g[src] rows for this core's edges (GPSIMD ucode gather;
     the table is split into 4 row-buckets so rows fit int16 indexing)
  4. segment-sum: one-hot selection matrix (DVE is_equal vs iota) contracted
     with the gathered message chunk on the TensorEngine, accumulated in PSUM
  5. epilogue: scale by dinv[dst] (+bias), relu for layer 1
The GCN norm dinv[src]*dinv[dst] factorizes into per-node scales: src side
into step 1, dst side into step 5.  Layer 1 keeps its segment-sum output
feature-major ([feat, node]) so layer 2's transform consumes it as lhsT
without a transpose; layer 1's dst-side dinv is folded into transform 2's
scale as dinv^2 (relu is positively homogeneous; requires b1 == 0, which
holds for this model -- nonzero b1 falls back to numpy).
"""

import os
import sys

import numpy as np

for _p in ("/opt/trn_rl_repo", "/root/.axon_site/_ro/trn_rl_repo"):
    if os.path.isdir(_p) and _p not in sys.path:
        sys.path.insert(0, _p)

N_NODES = 100000
N_EDGES = 1600000
D = 128
N_CORES = 8
NPC = 12500              # real nodes per core
TPC = 98                 # tiles per core
NPC_PAD = TPC * 128      # 12544
TABLE_ROWS = N_CORES * NPC_PAD   # 100352
N_BUCKETS = 4
BUCKET_ROWS = TABLE_ROWS // N_BUCKETS    # 25088 < 32768 (int16 reach)
SUPER = 4                # dst tiles per gather call (1 PSUM bank each)


def _prep(edge_index):
    e = np.asarray(edge_index)
    loops = np.arange(N_NODES, dtype=np.int64)
    src = np.concatenate([e[0].astype(np.int64), loops])
    dst = np.concatenate([e[1].astype(np.int64), loops])
    deg = np.bincount(dst, minlength=N_NODES).astype(np.float32)
    dinv = np.where(deg > 0, 1.0 / np.sqrt(deg), 0.0).astype(np.float32)

    core = dst // NPC
    local = dst % NPC
    tile = local // 128
    slot = local % 128
    src_row = (src // NPC) * NPC_PAD + (src % NPC)
    bucket = src_row // BUCKET_ROWS
    src_loc = src_row % BUCKET_ROWS

    # uniform chunk quota per (tile, bucket) across all cores/tiles
    ctb = (core * TPC + tile) * N_BUCKETS + bucket
    counts = np.bincount(ctb, minlength=N_CORES * TPC * N_BUCKETS)
    cnt_tb = counts.reshape(N_CORES * TPC, N_BUCKETS)
    Qb = tuple(int(v) for v in (cnt_tb.max(axis=0) + 127) // 128)
    Qtot = sum(Qb)
    nchunk = TPC * Qtot

    order = np.argsort(ctb, kind="stable")
    sl_s = src_loc[order]
    slot_s = slot[order]
    ctb_s = ctb[order]
    starts = np.zeros(len(counts) + 1, dtype=np.int64)
    np.cumsum(counts, out=starts[1:])
    pos = np.arange(len(ctb_s)) - starts[ctb_s]

    c_core = ctb_s // (TPC * N_BUCKETS)
    c_tile = (ctb_s // N_BUCKETS) % TPC
    c_buck = ctb_s % N_BUCKETS
    qoff = np.concatenate([[0], np.cumsum(Qb)])[:-1]
    chunk_in_tile = qoff[c_buck] + pos // 128
    part = pos % 128
    chunk_global = c_tile * Qtot + chunk_in_tile

    # token t = chunk*128 + part; pads keep idx 0 (valid row, S-masked)
    idx16 = np.zeros((N_CORES, nchunk * 128), dtype=np.int16)
    dl_all = np.full((N_CORES, 128, nchunk), -1.0, dtype=np.float32)
    t_glob = chunk_global * 128 + part
    idx16[c_core, t_glob] = sl_s.astype(np.int16)
    dl_all[c_core, part, chunk_global] = slot_s

    return dinv, idx16, dl_all, Qb


def _pack_idx_callorder(idx16, Qb):
    """Reorder the token stream into gather-call order (super, bucket,
    tile, chunk, part), then wrap for the int16 idx tile layout:
    token t -> [t % 16, t // 16], replicated across all 8 groups of 16
    partitions (dma_gather convention)."""
    Qtot = sum(Qb)
    qoff = np.concatenate([[0], np.cumsum(Qb)])[:-1]
    view = idx16.reshape(N_CORES, TPC, Qtot, 128)
    pieces = []
    for s0 in range(0, TPC, SUPER):
        ntiles = min(SUPER, TPC - s0)
        for b in range(N_BUCKETS):
            blk = view[:, s0:s0 + ntiles, qoff[b]:qoff[b] + Qb[b], :]
            pieces.append(blk.reshape(N_CORES, -1))
    one = np.concatenate(pieces, axis=1)
    n = one.shape[1]
    wrapped = one.reshape(N_CORES, n // 16, 16).transpose(0, 2, 1)
    return np.ascontiguousarray(
        np.broadcast_to(wrapped[:, None, :, :],
                        (N_CORES, 8, 16, n // 16)).reshape(
                            N_CORES, 128, n // 16))


_CACHE = {}


def _build(Qb):
    import concourse.bass as bass
    import concourse.bacc as bacc
    import concourse.mybir as mybir
    import concourse.tile as tile

    fp32 = mybir.dt.float32
    bf16 = mybir.dt.bfloat16
    i16 = mybir.dt.int16
    Qtot = sum(Qb)
    nchunk = TPC * Qtot
    ntok = nchunk * 128
    qoff = [0]
    for q in Qb:
        qoff.append(qoff[-1] + q)
    supers = [(s, min(SUPER, TPC - s)) for s in range(0, TPC, SUPER)]

    # call list: (super start, ntiles, bucket, token offset in call order)
    calls = []
    off = 0
    for (s0, ntiles) in supers:
        for b in range(N_BUCKETS):
            calls.append((s0, ntiles, b, off))
            off += ntiles * Qb[b] * 128
    assert off == ntok

    nc = bacc.Bacc(None)

    xT = nc.dram_tensor("xT", [128, NPC_PAD], bf16, kind="ExternalInput")
    w1 = nc.dram_tensor("w1", [128, 128], bf16, kind="ExternalInput")
    w2 = nc.dram_tensor("w2", [128, 128], bf16, kind="ExternalInput")
    b2r = nc.dram_tensor("b2r", [128, 128], fp32, kind="ExternalInput")
    dinv_col = nc.dram_tensor("dinv_col", [128, TPC], fp32,
                              kind="ExternalInput")
    dinv_col2 = nc.dram_tensor("dinv_col2", [128, TPC], fp32,
                               kind="ExternalInput")
    iota_in = nc.dram_tensor("iota", [128, 128], bf16, kind="ExternalInput")
    idx_in = nc.dram_tensor("idx", [128, ntok // 16], i16,
                            kind="ExternalInput")
    dl_in = nc.dram_tensor("dstloc", [128, nchunk], fp32,
                           kind="ExternalInput")
    out = nc.dram_tensor("out", [NPC_PAD, 128], fp32, kind="ExternalOutput")

    g1_shard = nc.dram_tensor("g1_shard", [NPC_PAD, 128], bf16)
    g1_full = nc.dram_tensor("g1_full", [TABLE_ROWS, 128], bf16,
                             addr_space="Shared")
    g2_shard = nc.dram_tensor("g2_shard", [NPC_PAD, 128], bf16)
    g2_full = nc.dram_tensor("g2_full", [TABLE_ROWS, 128], bf16,
                             addr_space="Shared")

    groups = [list(range(N_CORES))]

    with tile.TileContext(nc) as tc:
        with (
            tc.tile_pool(name="persist", bufs=1) as pp,
            tc.tile_pool(name="xpool", bufs=1) as xp,
            tc.tile_pool(name="gstage", bufs=1) as gp,
            tc.tile_pool(name="tok", bufs=2) as tokp,
            tc.tile_pool(name="sel", bufs=8) as sp,
            tc.tile_pool(name="tmp", bufs=4) as tp,
            tc.tile_pool(name="psum_t", bufs=2, space="PSUM") as pst,
            tc.tile_pool(name="psum_g", bufs=6, space="PSUM") as psg,
        ):
            w1_s = pp.tile([128, 128], bf16, tag="w1")
            w2_s = pp.tile([128, 128], bf16, tag="w2")
            b2_s = pp.tile([128, 128], fp32, tag="b2")
            dc_s = pp.tile([128, TPC], fp32, tag="dc")
            dc2_s = pp.tile([128, TPC], fp32, tag="dc2")
            io_s = pp.tile([128, 128], bf16, tag="iota")
            idx_s = pp.tile([128, ntok // 16], i16, tag="idx")
            dl_s = pp.tile([128, nchunk], fp32, tag="dl")
            h1T = pp.tile([128, NPC_PAD], bf16, tag="h1T")

            nc.sync.dma_start(out=w1_s[:], in_=w1[:, :])
            nc.sync.dma_start(out=w2_s[:], in_=w2[:, :])
            nc.sync.dma_start(out=b2_s[:], in_=b2r[:, :])
            nc.sync.dma_start(out=dc_s[:], in_=dinv_col[:, :])
            nc.sync.dma_start(out=dc2_s[:], in_=dinv_col2[:, :])
            nc.sync.dma_start(out=io_s[:], in_=iota_in[:, :])
            nc.sync.dma_start(out=idx_s[:], in_=idx_in[:, :])
            nc.sync.dma_start(out=dl_s[:], in_=dl_in[:, :])

            xT_s = xp.tile([128, NPC_PAD], bf16, tag="xT")
            nc.sync.dma_start(out=xT_s[:], in_=xT[:, :])

            sizes = sorted({nt * Qb[b] * 128
                            for (_, nt, b, _) in calls})
            size_regs = {n: nc.gpsimd.to_reg(n) for n in sizes}

            max_call_chunks = SUPER * max(Qb)
            tok_bufs = []
            for _ in range(2):
                tkb = tokp.tile([128, max_call_chunks, 128], bf16,
                                tag="tok", name="tok")
                # stale pad slots must stay finite (0 * NaN = NaN)
                nc.vector.memset(tkb[:], 0)
                tok_bufs.append(tkb)

            def transform(src_sbuf, w_sbuf, g_dram, scale_col):
                gst = gp.tile([128, TPC * 128], bf16, tag="gst")
                for t in range(TPC):
                    ps = pst.tile([128, 128], fp32, tag="pt")
                    nc.tensor.matmul(
                        out=ps[:],
                        lhsT=src_sbuf[:, bass.ts(t, 128)],
                        rhs=w_sbuf[:],
                        start=True, stop=True,
                    )
                    nc.scalar.activation(
                        out=gst[:, bass.ts(t, 128)], in_=ps[:],
                        func=mybir.ActivationFunctionType.Copy,
                        scale=scale_col[:, t:t + 1],
                    )
                g_view = g_dram.ap().rearrange("(t p) f -> p t f", p=128)
                nc.sync.dma_start(out=g_view, in_=gst[:].rearrange(
                    "p (t f) -> p t f", f=128))

            def gather_phase(g_full_t, layer):
                cno = 0
                banks = {}
                for (s0, ntiles, b, off) in calls:
                    n_idx = ntiles * Qb[b] * 128
                    tok = tok_bufs[cno % 2]
                    cno += 1
                    nc.gpsimd.dma_gather(
                        out_ap=tok[:, :ntiles * Qb[b], :],
                        in_ap=g_full_t[b * BUCKET_ROWS:
                                       (b + 1) * BUCKET_ROWS, :],
                        idxs_ap=idx_s[:, off // 16:(off + n_idx) // 16],
                        num_idxs=n_idx,
                        num_idxs_reg=size_regs[n_idx],
                        elem_size=128,
                        single_packet=False,
                    )
                    if b == 0:
                        for bi in range(ntiles):
                            banks[bi] = psg.tile([128, 128], fp32,
                                                 tag="pg", name="pg")
                    for jj in range(ntiles * Qb[b]):
                        tloc = jj // Qb[b]
                        q = jj % Qb[b]
                        t = s0 + tloc
                        cglob = t * Qtot + qoff[b] + q
                        first = (b == 0 and q == 0)
                        last = (b == N_BUCKETS - 1 and q == Qb[b] - 1)
                        S = sp.tile([128, 128], bf16, tag="sel", name="sel")
                        nc.vector.tensor_scalar(
                            out=S[:], in0=io_s[:],
                            scalar1=dl_s[:, cglob:cglob + 1], scalar2=None,
                            op0=mybir.AluOpType.is_equal,
                        )
                        pslice = banks[tloc][:]
                        if layer == 1:
                            nc.tensor.matmul(out=pslice, lhsT=tok[:, jj, :],
                                             rhs=S[:],
                                             start=first, stop=last)
                        else:
                            nc.tensor.matmul(out=pslice, lhsT=S[:],
                                             rhs=tok[:, jj, :],
                                             start=first, stop=last)
                        if last:
                            if layer == 1:
                                nc.scalar.activation(
                                    out=h1T[:, bass.ts(t, 128)], in_=pslice,
                                    func=mybir.ActivationFunctionType.Relu)
                            else:
                                tm2 = tp.tile([128, 128], fp32, tag="tm2")
                                nc.scalar.activation(
                                    out=tm2[:], in_=pslice,
                                    func=mybir.ActivationFunctionType.Copy,
                                    scale=dc_s[:, t:t + 1])
                                tm3 = tp.tile([128, 128], fp32, tag="tm3")
                                nc.vector.tensor_tensor(
                                    out=tm3[:], in0=tm2[:], in1=b2_s[:],
                                    op=mybir.AluOpType.add)
                                nc.sync.dma_start(
                                    out=out[bass.ts(t, 128), :], in_=tm3[:])

            transform(xT_s, w1_s, g1_shard, dc_s)
            nc.gpsimd.collective_compute(
                "AllGather", mybir.AluOpType.bypass,
                replica_groups=groups,
                ins=[g1_shard.ap().opt()],
                outs=[g1_full.ap().opt()],
            )
            gather_phase(g1_full, layer=1)

            transform(h1T, w2_s, g2_shard, dc2_s)
            nc.gpsimd.collective_compute(
                "AllGather", mybir.AluOpType.bypass,
                replica_groups=groups,
                ins=[g2_shard.ap().opt()],
                outs=[g2_full.ap().opt()],
            )
            gather_phase(g2_full, layer=2)

    nc.finalize()
    return nc


def kernel(x, edge_index, W1, b1, W2, b2):
    import ml_dtypes

    x = np.asarray(x, dtype=np.float32)
    W1 = np.asarray(W1, dtype=np.float32)
    b1 = np.asarray(b1, dtype=np.float32)
    W2 = np.asarray(W2, dtype=np.float32)
    b2 = np.asarray(b2, dtype=np.float32)

    if np.any(b1 != 0.0):
        return _kernel_numpy(x, edge_index, W1, b1, W2, b2)

    ekey = hash(np.asarray(edge_index)[:, ::65537].tobytes())
    if "prep" not in _CACHE or _CACHE.get("ekey") != ekey:
        dinv, idx16, dl_all, Qb = _prep(edge_index)
        idx_packed = _pack_idx_callorder(idx16, Qb)
        _CACHE["prep"] = (dinv, idx_packed, dl_all, Qb)
        _CACHE["ekey"] = ekey
    dinv, idx_packed, dl_all, Qb = _CACHE["prep"]

    if _CACHE.get("nc_key") != Qb:
        _CACHE["nc"] = _build(Qb)
        _CACHE["nc_key"] = Qb
    nc = _CACHE["nc"]

    bf = ml_dtypes.bfloat16
    iota = np.broadcast_to(np.arange(128, dtype=np.float32), (128, 128))

    in_maps = []
    for c in range(N_CORES):
        lo = c * NPC
        xs = np.zeros((NPC_PAD, 128), np.float32)
        xs[:NPC] = x[lo:lo + NPC]
        dv = np.zeros(NPC_PAD, np.float32)
        dv[:NPC] = dinv[lo:lo + NPC]
        in_maps.append({
            "xT": np.ascontiguousarray(xs.T).astype(bf),
            "w1": W1.astype(bf),
            "w2": W2.astype(bf),
            "b2r": np.ascontiguousarray(
                np.broadcast_to(b2, (128, 128))).astype(np.float32),
            "dinv_col": np.ascontiguousarray(dv.reshape(TPC, 128).T),
            "dinv_col2": np.ascontiguousarray(
                (dv * dv).reshape(TPC, 128).T),
            "iota": iota.astype(bf),
            "idx": idx_packed[c],
            "dstloc": dl_all[c],
        })

    from concourse import bass_utils
    trace = bool(os.environ.get("GCN_TRACE"))
    if trace:
        try:
            from antenv.axon_hooks import get_axon_ntff_profile_hook  # noqa
        except Exception:
            trace = False   # NTFF tracing unavailable in this container
    res = bass_utils.run_bass_kernel_spmd(
        nc, in_maps, list(range(N_CORES)), trace=trace)
    _CACHE["exec_time_ns"] = res.exec_time_ns
    _CACHE["mean_exec_time_ns"] = res.mean_exec_time_ns
    _CACHE["profile"] = res.instructions_and_trace
    outs = [np.asarray(res.results[c]["out"][:NPC], np.float32)
            for c in range(N_CORES)]
    return np.concatenate(outs, axis=0)


def _kernel_numpy(x, edge_index, W1, b1, W2, b2):
    loops = np.arange(N_NODES)
    src = np.concatenate([np.asarray(edge_index[0]), loops])
    dst = np.concatenate([np.asarray(edge_index[1]), loops])
    deg = np.bincount(dst, minlength=N_NODES).astype(np.float32)
    dinv = np.where(deg > 0, 1 / np.sqrt(deg), 0)
    norm = (dinv[src] * dinv[dst]).astype(np.float32)
    order = np.argsort(dst, kind="stable")
    s_s, n_s = src[order], norm[order]
    counts = np.bincount(dst, minlength=N_NODES)
    starts = np.zeros(N_NODES, dtype=np.int64)
    np.cumsum(counts[:-1], out=starts[1:])

    def conv(h, W, b):
        hw = h @ W
        msg = hw[s_s] * n_s[:, None]
        o = np.add.reduceat(msg, starts, axis=0)
        o[counts == 0] = 0.0
        return o + b

    h = np.maximum(conv(x, W1, b1), 0.0)
    return conv(h, W2, b2).astype(np.float32)
